# revision 1
# baseline (speedup 1.0000x reference)
"""Trainium2 Bass kernel for nn_FFT_TREND (B=32, N=256, T=2048, K=5).

Pure data-parallel over 8 NeuronCores: each core handles 4 samples.

Per-core pipeline (all on device):
  A. Load x, even/odd fold along t, PE-transpose to [t, ch] layout.
  B. Real DFT via fp32 matmuls (folded: cos on xe, sin on xo), |X| magnitude,
     channel-mean via Sqrt+accum, top-5 bins via max/max_index.
  C. Per (sample, kernel-size): moving average via extended cumsum array G
     (affine tails encode replicate padding), data-dependent shifts done as
     dynamic-slice reads with register offsets looked up from host-built
     tables; branchless rank-1 correction handles kernel sizes > 2T.
"""
import sys
sys.path.insert(0, "/opt/trn_rl_repo")
import os
import numpy as np

import concourse.bacc as bacc
import concourse.mybir as mybir
from concourse.bass import ds
from concourse.expressions import smin
from concourse.bass_utils import run_bass_kernel_spmd
from concourse.tile import TileContext

P = 128
B, N, T, KTOP = 32, 256, 2048, 5
FS = 120.0
NCORES = 8
BL = B // NCORES          # 4 samples per core
NBLK = N // P             # 2 channel blocks
NBINS = 1024              # bins 1..1024 (DC killed)
KC_E = 9                  # xe t-chunks (t = 0..1151, data 0..1024)
KC_O = 8                  # xo t-chunks (t = 0..1023)
KCHT = KC_E + KC_O        # 17
UW = KCHT * P             # 2176 cols per (sample, block) unit in xT
GW = 3 * T + 1            # 6145 cols in extended-cumsum array G
dt = mybir.dt

_cache = {}


def _host_consts():
    if "consts" in _cache:
        return _cache["consts"]
    pos = np.arange(NBINS)
    idxf = (pos + 1).astype(np.float32)
    freq = idxf * np.float32(FS / T)            # exact fp32 (FS/T = 15/256)
    k = np.floor(np.float32(T) / freq).astype(np.int64)  # replicates reference
    p = (k - 1) // 2
    q = k - p                      # hi shift: p+1 odd k, p+2 even k
    pc = np.minimum(p, T - 1)
    qc = np.minimum(q, T)
    dl = (p - pc).astype(np.float64)
    dh = (q - qc).astype(np.float64)
    invk = (1.0 / k.astype(np.float32)).astype(np.float32)
    consts = dict(
        hi_t=(2048 + qc).astype(np.int32)[None, :],
        lo_t=(2048 - pc).astype(np.int32)[None, :],
        last_t=np.where(k % 2 == 0, 2046, 2047).astype(np.int32)[None, :],
        invk_t=np.tile(invk[None, :], (P, 1)),
        dlinvk_t=np.tile((dl / k).astype(np.float32)[None, :16], (P, 1)),
        dhinvk_t=np.tile((dh / k).astype(np.float32)[None, :16], (P, 1)),
        ramp=np.tile(np.arange(1, T + 1, dtype=np.float32)[None, :], (P, 1)),
        ident=np.eye(P, dtype=np.float32),
    )
    # DFT matrices (folded real DFT, bins 1..1024)
    tt = np.arange(KC_E * P, dtype=np.float64)          # 0..1151
    bins = np.arange(1, NBINS + 1, dtype=np.float64)
    ang = 2.0 * np.pi / T * tt[:, None] * bins[None, :]
    wc = np.cos(ang)
    wc[tt > 1024, :] = 0.0
    ws = np.sin(ang[:KC_O * P])                          # t = 0..1023
    # layout [kc, g, 128, 256] with col = fi*128 + j, fc = 2g+fi, bin = fc*128+j+1
    # wc rows are t = kc*128 + r, cols f = (2g+fi)*128 + j
    # target wc4[kc, g, r, fi*128+j]
    wc4 = (wc.reshape(KC_E, P, 4, 2, P).transpose(0, 2, 1, 3, 4)
           .reshape(KC_E, 4, P, 2 * P))
    ws4 = (ws.reshape(KC_O, P, 4, 2, P).transpose(0, 2, 1, 3, 4)
           .reshape(KC_O, 4, P, 2 * P))
    consts["wc_t"] = np.ascontiguousarray(wc4, dtype=np.float32)
    consts["ws_t"] = np.ascontiguousarray(ws4, dtype=np.float32)
    _cache["consts"] = consts
    return consts


USE_POOL_TT = os.environ.get("KERNEL_POOL_TT", "0") == "1"


def _build():
    if "nc" in _cache:
        return _cache["nc"]
    nc = bacc.Bacc("TRN2", target_bir_lowering=False, debug=False)
    DVE = [mybir.EngineType.DVE]
    A = mybir.AluOpType
    AF = mybir.ActivationFunctionType

    x_t = nc.dram_tensor("x", (BL, N, T), dt.float32, kind="ExternalInput").ap()
    wc_t = nc.dram_tensor("wc_t", (KC_E, 4, P, 2 * P), dt.float32, kind="ExternalInput").ap()
    ws_t = nc.dram_tensor("ws_t", (KC_O, 4, P, 2 * P), dt.float32, kind="ExternalInput").ap()
    ramp_t = nc.dram_tensor("ramp", (P, T), dt.float32, kind="ExternalInput").ap()
    ident_t = nc.dram_tensor("ident", (P, P), dt.float32, kind="ExternalInput").ap()
    hi_t = nc.dram_tensor("hi_t", (1, NBINS), dt.int32, kind="ExternalInput").ap()
    lo_t = nc.dram_tensor("lo_t", (1, NBINS), dt.int32, kind="ExternalInput").ap()
    last_t = nc.dram_tensor("last_t", (1, NBINS), dt.int32, kind="ExternalInput").ap()
    invk_t = nc.dram_tensor("invk_t", (P, NBINS), dt.float32, kind="ExternalInput").ap()
    dlinvk_t = nc.dram_tensor("dlinvk_t", (P, 16), dt.float32, kind="ExternalInput").ap()
    dhinvk_t = nc.dram_tensor("dhinvk_t", (P, 16), dt.float32, kind="ExternalInput").ap()
    out_t = nc.dram_tensor("out", (BL, N, KTOP, T), dt.float32, kind="ExternalOutput").ap()

    with TileContext(nc) as tc:
        with tc.tile_pool(name="const", bufs=1) as cpool, \
             tc.tile_pool(name="xT", bufs=1) as xTpool, \
             tc.tile_pool(name="small", bufs=1) as spool:
            identt = cpool.tile([P, P], dt.float32)
            nc.sync.dma_start(identt, ident_t)
            rampt = cpool.tile([P, T], dt.float32)
            nc.sync.dma_start(rampt, ramp_t)
            hit = cpool.tile([1, NBINS], dt.int32)
            nc.sync.dma_start(hit, hi_t)
            lot = cpool.tile([1, NBINS], dt.int32)
            nc.sync.dma_start(lot, lo_t)
            lastt = cpool.tile([1, NBINS], dt.int32)
            nc.sync.dma_start(lastt, last_t)
            invkt = cpool.tile([P, NBINS], dt.float32)
            nc.sync.dma_start(invkt, invk_t)
            dlinvkt = cpool.tile([P, 16], dt.float32)
            nc.sync.dma_start(dlinvkt, dlinvk_t)
            dhinvkt = cpool.tile([P, 16], dt.float32)
            nc.sync.dma_start(dhinvkt, dhinvk_t)

            xTt = xTpool.tile([P, 2 * BL * UW], dt.float32)
            xTr = xTt[:].rearrange("p (u c) -> p u c", c=UW)

            # ---------------- Phase A: fold + transpose ----------------
            with tc.tile_pool(name="xnat", bufs=2) as xnp, \
                 tc.tile_pool(name="fold", bufs=2) as fp, \
                 tc.tile_pool(name="tpps", bufs=2, space="PSUM") as tpp:
                for b in range(BL):
                    for blk in range(NBLK):
                        u = b * NBLK + blk
                        xn = xnp.tile([P, T], dt.float32, tag="xn")
                        nc.sync.dma_start(xn, x_t[b, blk * P:(blk + 1) * P, :])
                        xe = fp.tile([P, KC_E * P], dt.float32, tag="xe")
                        xo = fp.tile([P, KC_O * P], dt.float32, tag="xo")
                        nc.vector.tensor_tensor(
                            xe[:, 1:1024], xn[:, 1:1024], xn[:, 2047:1024:-1], A.add)
                        nc.vector.tensor_copy(xe[:, 0:1], xn[:, 0:1])
                        nc.vector.tensor_copy(xe[:, 1024:1025], xn[:, 1024:1025])
                        nc.vector.memset(xe[:, 1025:KC_E * P], 0.0)
                        nc.vector.tensor_tensor(
                            xo[:, 1:1024], xn[:, 1:1024], xn[:, 2047:1024:-1], A.subtract)
                        nc.vector.memset(xo[:, 0:1], 0.0)
                        for grp in range(5):
                            c0 = grp * 4
                            ncks = min(4, KCHT - c0)
                            tp = tpp.tile([P, 512], dt.float32, tag="tp")
                            for ci in range(ncks):
                                c = c0 + ci
                                src = (xe[:, c * P:(c + 1) * P] if c < KC_E
                                       else xo[:, (c - KC_E) * P:(c - KC_E + 1) * P])
                                nc.tensor.transpose(
                                    tp[:, ci * P:(ci + 1) * P], src, identt)
                            nc.scalar.activation(
                                xTt[:, u * UW + c0 * P: u * UW + c0 * P + ncks * P],
                                tp[:, 0:ncks * P], AF.Copy)

            # ---------------- Phases B+C interleaved ----------------
            # DFT runs in two 2-sample passes; each pass's moving-average work
            # is emitted immediately after it so its DVE/DMA overlaps the next
            # pass's matmuls instead of queuing behind them.
            idxrows = []
            with tc.tile_pool(name="wdma", bufs=int(os.environ.get("BUF_W", "3"))) as wp, \
                 tc.tile_pool(name="dftps", bufs=1, space="PSUM") as dpp, \
                 tc.tile_pool(name="mtps", bufs=1, space="PSUM") as mtp, \
                 tc.tile_pool(name="sq", bufs=int(os.environ.get("BUF_SQ", "2"))) as sqp, \
                 tc.tile_pool(name="xnat2", bufs=int(os.environ.get("BUF_XN", "2"))) as xnp2, \
                 tc.tile_pool(name="Gp", bufs=2) as gp, \
                 tc.tile_pool(name="colp", bufs=2) as clp, \
                 tc.tile_pool(name="magp", bufs=2) as mgp, \
                 tc.tile_pool(name="comb", bufs=int(os.environ.get("BUF_COMB", "2"))) as cbp:

                _plan = os.environ.get("KERNEL_PLAN", "2,2")
                PASSES = []           # (first_sample, n_samples)
                _s = 0
                for _n in [int(v) for v in _plan.split(",")]:
                    PASSES.append((_s, _n))
                    _s += _n
                assert _s == BL

                def emit_dft_half(half):
                    b0, SP = PASSES[half]
                    u0 = b0 * 2
                    magsum = mgp.tile([P, 8 * SP], dt.float32, tag="magsum", name="magsum")
                    for g in range(4):
                        psC = []
                        psS = []
                        for i in range(2):
                            psc_i = dpp.tile([P, 256 * SP], dt.float32, tag=f"psc{i}")
                            pss_i = dpp.tile([P, 256 * SP], dt.float32, tag=f"pss{i}")
                            psC.append(psc_i)
                            psS.append(pss_i)
                        for kc in range(KC_E):
                            wct = wp.tile([P, 2 * P], dt.float32, tag="wc")
                            nc.sync.dma_start(wct, wc_t[kc, g])
                            wst = None
                            if kc < KC_O:
                                wst = wp.tile([P, 2 * P], dt.float32, tag="ws")
                                nc.sync.dma_start(wst, ws_t[kc, g])
                            for fi in range(2):
                                rhs_e = xTr[:, u0:u0 + 2 * SP, kc * P:(kc + 1) * P]
                                nc.tensor.matmul(
                                    psC[fi], wct[:, fi * P:(fi + 1) * P], rhs_e,
                                    start=(kc == 0), stop=(kc == KC_E - 1),
                                    skip_group_check=True)
                                if kc < KC_O:
                                    rhs_o = xTr[:, u0:u0 + 2 * SP,
                                                (KC_E + kc) * P:(KC_E + kc + 1) * P]
                                    nc.tensor.matmul(
                                        psS[fi], wst[:, fi * P:(fi + 1) * P], rhs_o,
                                        start=(kc == 0), stop=(kc == KC_O - 1),
                                        skip_group_check=True)
                        for fi in range(2):
                            fc = 2 * g + fi
                            sqc = sqp.tile([P, 256 * SP], dt.float32, tag="sqc")
                            sqs = sqp.tile([P, 256 * SP], dt.float32, tag="sqs")
                            scr = sqp.tile([P, 256], dt.float32, tag="scr")
                            nc.scalar.activation(sqc, psC[fi], AF.Square)
                            nc.scalar.activation(sqs, psS[fi], AF.Square)
                            nc.vector.tensor_tensor(sqc, sqc, sqs, A.add)
                            for bh in range(SP):
                                nc.scalar.activation(
                                    scr, sqc[:, bh * 256:(bh + 1) * 256], AF.Sqrt,
                                    accum_out=magsum[:, fc * SP + bh: fc * SP + bh + 1])
                    mag_h = mgp.tile([SP, NBINS], dt.float32, tag="mag_h", name="mag_h")
                    mt = mtp.tile([8 * SP, P], dt.float32, tag="mt", name="mt")
                    nc.tensor.transpose(mt, magsum[:, 0:8 * SP], identt)
                    mtsb = mgp.tile([8 * SP, P], dt.float32, tag="mtsb", name="mtsb")
                    nc.scalar.activation(mtsb, mt, AF.Copy)
                    for fc in range(8):
                        nc.sync.dma_start(
                            mag_h[0:SP, fc * P:(fc + 1) * P],
                            mtsb[fc * SP:fc * SP + SP, :])
                    mx = mgp.tile([SP, 8], dt.float32, tag="mx", name="mx")
                    mi = mgp.tile([SP, 8], dt.uint32, tag="mi", name="mi")
                    nc.vector.max(out=mx, in_=mag_h)
                    nc.vector.max_index(mi, mx, mag_h)
                    idxrow = mgp.tile([1, 8 * SP], dt.uint32, tag="idxrow", name="idxrow")
                    nc.sync.dma_start(idxrow, mi)
                    idxrows.append(idxrow)

                def emit_sample_C(b):
                    Gs, cols = [], []
                    for blk in range(NBLK):
                        xn = xnp2.tile([P, T], dt.float32, tag="xn2", name="xn2")
                        nc.sync.dma_start(xn, x_t[b, blk * P:(blk + 1) * P, :])
                        G = gp.tile([P, GW], dt.float32, tag="G", name="G")
                        cl = clp.tile([P, 8], dt.float32, tag=f"cols{blk}",
                                      name=f"cols{blk}")
                        nc.vector.tensor_copy(cl[:, 0:1], xn[:, 0:1])
                        nc.vector.tensor_copy(cl[:, 1:2], xn[:, 2047:2048])
                        nc.vector.tensor_scalar_mul(cl[:, 2:3], cl[:, 0:1], -2049.0)
                        nc.vector.tensor_tensor_scan(
                            G[:, T + 1:2 * T + 1], xn, xn, 0.0, A.add, A.bypass)
                        nc.vector.memset(G[:, T:T + 1], 0.0)
                        nc.scalar.activation(
                            G[:, 0:T], rampt, AF.Identity,
                            bias=cl[:, 2:3], scale=cl[:, 0:1])
                        nc.scalar.activation(
                            G[:, 2 * T + 1:GW], rampt, AF.Identity,
                            bias=G[:, 2 * T:2 * T + 1], scale=cl[:, 1:2])
                        Gs.append(G)
                        cols.append(cl)
                    _half = max(h for h, (s0, _) in enumerate(PASSES) if s0 <= b)
                    _boff = b - PASSES[_half][0]
                    for kk in range(KTOP):
                        j = _boff * 8 + kk
                        _eng = (DVE + [mybir.EngineType.Pool]
                                if USE_POOL_TT else DVE)
                        idx = nc.values_load(
                            idxrows[_half][0:1, j:j + 1], engines=_eng,
                            min_val=0, max_val=NBINS - 1,
                            skip_runtime_bounds_check=True)
                        hi_s = nc.values_load(
                            hit[0:1, ds(idx, 1)], engines=_eng,
                            min_val=2065, max_val=4096,
                            skip_runtime_bounds_check=True)
                        lo_s = nc.values_load(
                            lot[0:1, ds(idx, 1)], engines=_eng,
                            min_val=1, max_val=2032,
                            skip_runtime_bounds_check=True)
                        last = nc.values_load(
                            lastt[0:1, ds(idx, 1)], engines=DVE,
                            min_val=2046, max_val=2047,
                            skip_runtime_bounds_check=True)
                        for blk in range(NBLK):
                            G, cl = Gs[blk], cols[blk]
                            comb = cbp.tile([P, T], dt.float32, tag="comb",
                                            name="comb")
                            _sub_eng = (nc.gpsimd if (USE_POOL_TT and blk == 1)
                                        else nc.vector)
                            _sub_eng.tensor_tensor(
                                comb, G[:, ds(hi_s, T)], G[:, ds(lo_s, T)],
                                A.subtract)
                            nc.vector.tensor_tensor(
                                comb[:, T - 1:T],
                                G[:, ds(hi_s + last, 1)], G[:, ds(lo_s + last, 1)],
                                A.subtract)
                            idxc = smin(idx, 15)   # dl=dh=0 for idx >= 9
                            nc.vector.tensor_scalar_mul(
                                cl[:, 4:5], cl[:, 0:1], dlinvkt[:, ds(idxc, 1)])
                            nc.vector.scalar_tensor_tensor(
                                cl[:, 5:6], cl[:, 1:2], dhinvkt[:, ds(idxc, 1)],
                                cl[:, 4:5], A.mult, A.add)
                            nc.vector.tensor_scalar(
                                comb, comb, invkt[:, ds(idx, 1)], cl[:, 5:6],
                                A.mult, A.add)
                            nc.sync.dma_start(
                                out_t[b, blk * P:(blk + 1) * P, kk, :], comb)

                for half, (s0, sp_n) in enumerate(PASSES):
                    emit_dft_half(half)
                    for bh in range(sp_n):
                        emit_sample_C(s0 + bh)

    nc.compile()
    _cache["nc"] = nc
    return nc


def _in_maps(x):
    consts = _host_consts()
    x = np.ascontiguousarray(x, dtype=np.float32)
    return [dict(x=x[c * BL:(c + 1) * BL], **consts) for c in range(NCORES)]


def _run(x, **kw):
    nc = _build()
    return run_bass_kernel_spmd(nc, _in_maps(x), core_ids=list(range(NCORES)), **kw)


def _get_exec():
    """Cached PJRT executable over the 8 axon cores (mirrors
    bass2jax.run_bass_via_pjrt's multi-core branch, but jit-cached)."""
    if "exec" in _cache:
        return _cache["exec"]
    import jax
    from jax.sharding import Mesh, PartitionSpec
    from jax.experimental.shard_map import shard_map
    import concourse.bass2jax as b2j
    import concourse.mybir as mybir_

    b2j.install_neuronx_cc_hook()
    nc = _build()
    pname = nc.partition_id_tensor.name if nc.partition_id_tensor else None
    in_names, out_names, out_avals, zero_shapes = [], [], [], []
    for alloc in nc.m.functions[0].allocations:
        if not isinstance(alloc, mybir_.MemoryLocationSet):
            continue
        name = alloc.memorylocations[0].name
        if alloc.kind == "ExternalInput":
            if name != pname:
                in_names.append(name)
        elif alloc.kind == "ExternalOutput":
            shape = tuple(alloc.tensor_shape)
            np_dt = mybir_.dt.np(alloc.dtype)
            out_names.append(name)
            out_avals.append(jax.core.ShapedArray(shape, np_dt))
            zero_shapes.append((shape, np_dt))
    n_params = len(in_names)
    all_in_names = in_names + out_names
    if pname is not None:
        all_in_names = all_in_names + [pname]

    def _body(*args):
        operands = list(args)
        if pname is not None:
            operands.append(b2j.partition_id_tensor())
        outs = b2j._bass_exec_p.bind(
            *operands,
            out_avals=tuple(out_avals),
            in_names=tuple(all_in_names),
            out_names=tuple(out_names),
            lowering_input_output_aliases=(),
            sim_require_finite=True,
            sim_require_nnan=True,
            nc=nc,
        )
        return tuple(outs)

    devices = jax.devices()[:NCORES]
    mesh = Mesh(np.asarray(devices), ("core",))
    nio = n_params + len(out_names)
    sharded = jax.jit(
        shard_map(_body, mesh=mesh,
                  in_specs=(PartitionSpec("core"),) * nio,
                  out_specs=(PartitionSpec("core"),) * len(out_names),
                  check_rep=False),
        donate_argnums=tuple(range(n_params, nio)),
        keep_unused=True,
    )
    ex = dict(sharded=sharded, in_names=in_names, out_names=out_names,
              out_avals=out_avals, zero_shapes=zero_shapes, mesh=mesh)
    _cache["exec"] = ex
    return ex


def _concat_inputs(x):
    ex = _get_exec()
    maps = _in_maps(x)
    return [np.concatenate([maps[c][n] for c in range(NCORES)], axis=0)
            for n in ex["in_names"]]


def _make_zeros(on_device=False):
    ex = _get_exec()
    if on_device:
        import jax.numpy as jnp
        from jax.sharding import NamedSharding, PartitionSpec
        sh = NamedSharding(ex["mesh"], PartitionSpec("core"))
        return [jnp.zeros((NCORES * s[0], *s[1:]), d, device=sh)
                for s, d in ex["zero_shapes"]]
    return [np.zeros((NCORES * s[0], *s[1:]), d) for s, d in ex["zero_shapes"]]


def kernel(x):
    try:
        ex = _get_exec()
        outs = ex["sharded"](*_concat_inputs(x), *_make_zeros())
        out = np.asarray(outs[ex["out_names"].index("out")])
        return out.reshape(NCORES, BL, N, KTOP, T).reshape(B, N, KTOP, T)
    except Exception:
        res = _run(x)
        return np.concatenate([res.results[c]["out"] for c in range(NCORES)],
                              axis=0)



# revision 3
# speedup vs baseline: 115.5240x; 115.5240x over previous
"""Trainium2 Bass kernel for nn_FFT_TREND (B=32, N=256, T=2048, K=5).

Pure data-parallel over 8 NeuronCores: each core handles 4 samples.

Per-core pipeline (all on device):
  A. Load x, even/odd fold along t, PE-transpose to [t, ch] layout.
  B. Real DFT via fp32 matmuls (folded: cos on xe, sin on xo), |X| magnitude,
     channel-mean via Sqrt+accum, top-5 bins via max/max_index.
  C. Per (sample, kernel-size): moving average via extended cumsum array G
     (affine tails encode replicate padding), data-dependent shifts done as
     dynamic-slice reads with register offsets looked up from host-built
     tables; branchless rank-1 correction handles kernel sizes > 2T.
"""
import sys
sys.path.insert(0, "/opt/trn_rl_repo")
import os
import numpy as np

import concourse.bacc as bacc
import concourse.mybir as mybir
from concourse.bass import ds
from concourse.expressions import smin
from concourse.bass_utils import run_bass_kernel_spmd
from concourse.tile import TileContext

P = 128
B, N, T, KTOP = 32, 256, 2048, 5
FS = 120.0
NCORES = 8
BL = B // NCORES          # 4 samples per core
NBLK = N // P             # 2 channel blocks
NBINS = 1024              # bins 1..1024 (DC killed)
KC_E = 9                  # xe t-chunks (t = 0..1151, data 0..1024)
KC_O = 8                  # xo t-chunks (t = 0..1023)
KCHT = KC_E + KC_O        # 17
UW = KCHT * P             # 2176 cols per (sample, block) unit in xT
GW = 3 * T + 1            # 6145 cols in extended-cumsum array G
dt = mybir.dt

_cache = {}


def _host_consts():
    if "consts" in _cache:
        return _cache["consts"]
    pos = np.arange(NBINS)
    idxf = (pos + 1).astype(np.float32)
    freq = idxf * np.float32(FS / T)            # exact fp32 (FS/T = 15/256)
    k = np.floor(np.float32(T) / freq).astype(np.int64)  # replicates reference
    p = (k - 1) // 2
    q = k - p                      # hi shift: p+1 odd k, p+2 even k
    pc = np.minimum(p, T - 1)
    qc = np.minimum(q, T)
    dl = (p - pc).astype(np.float64)
    dh = (q - qc).astype(np.float64)
    invk = (1.0 / k.astype(np.float32)).astype(np.float32)
    consts = dict(
        hi_t=(2048 + qc).astype(np.int32)[None, :],
        lo_t=(2048 - pc).astype(np.int32)[None, :],
        last_t=np.where(k % 2 == 0, 2046, 2047).astype(np.int32)[None, :],
        invk_t=np.tile(invk[None, :], (P, 1)),
        dlinvk_t=np.tile((dl / k).astype(np.float32)[None, :16], (P, 1)),
        dhinvk_t=np.tile((dh / k).astype(np.float32)[None, :16], (P, 1)),
        ramp=np.tile(np.arange(1, T + 1, dtype=np.float32)[None, :], (P, 1)),
        ident=np.eye(P, dtype=np.float32),
    )
    # DFT matrices (folded real DFT, bins 1..1024)
    tt = np.arange(KC_E * P, dtype=np.float64)          # 0..1151
    bins = np.arange(1, NBINS + 1, dtype=np.float64)
    ang = 2.0 * np.pi / T * tt[:, None] * bins[None, :]
    wc = np.cos(ang)
    wc[tt > 1024, :] = 0.0
    ws = np.sin(ang[:KC_O * P])                          # t = 0..1023
    # layout [kc, g, 128, 256] with col = fi*128 + j, fc = 2g+fi, bin = fc*128+j+1
    # wc rows are t = kc*128 + r, cols f = (2g+fi)*128 + j
    # target wc4[kc, g, r, fi*128+j]
    wc4 = (wc.reshape(KC_E, P, 4, 2, P).transpose(0, 2, 1, 3, 4)
           .reshape(KC_E, 4, P, 2 * P))
    ws4 = (ws.reshape(KC_O, P, 4, 2, P).transpose(0, 2, 1, 3, 4)
           .reshape(KC_O, 4, P, 2 * P))
    consts["wc_t"] = np.ascontiguousarray(wc4, dtype=np.float32)
    consts["ws_t"] = np.ascontiguousarray(ws4, dtype=np.float32)
    _cache["consts"] = consts
    return consts


USE_POOL_TT = os.environ.get("KERNEL_POOL_TT", "0") == "1"


def _build():
    if "nc" in _cache:
        return _cache["nc"]
    consts = _host_consts()
    nc = bacc.Bacc("TRN2", target_bir_lowering=False, debug=False)
    DVE = [mybir.EngineType.DVE]
    A = mybir.AluOpType
    AF = mybir.ActivationFunctionType

    x_t = nc.dram_tensor("x", (BL, N, T), dt.float32, kind="ExternalInput").ap()
    wc_t = nc.inline_tensor(consts["wc_t"], name="wc_t").ap()
    ws_t = nc.inline_tensor(consts["ws_t"], name="ws_t").ap()
    ramp_t = nc.inline_tensor(consts["ramp"], name="ramp").ap()
    ident_t = nc.inline_tensor(consts["ident"], name="ident").ap()
    hi_t = nc.inline_tensor(consts["hi_t"], name="hi_t").ap()
    lo_t = nc.inline_tensor(consts["lo_t"], name="lo_t").ap()
    last_t = nc.inline_tensor(consts["last_t"], name="last_t").ap()
    invk_t = nc.inline_tensor(consts["invk_t"], name="invk_t").ap()
    dlinvk_t = nc.inline_tensor(consts["dlinvk_t"], name="dlinvk_t").ap()
    dhinvk_t = nc.inline_tensor(consts["dhinvk_t"], name="dhinvk_t").ap()
    out_t = nc.dram_tensor("out", (BL, N, KTOP, T), dt.float32, kind="ExternalOutput").ap()

    with TileContext(nc) as tc:
        with tc.tile_pool(name="const", bufs=1) as cpool, \
             tc.tile_pool(name="xT", bufs=1) as xTpool, \
             tc.tile_pool(name="small", bufs=1) as spool:
            identt = cpool.tile([P, P], dt.float32)
            nc.sync.dma_start(identt, ident_t)
            rampt = cpool.tile([P, T], dt.float32)
            nc.sync.dma_start(rampt, ramp_t)
            hit = cpool.tile([1, NBINS], dt.int32)
            nc.sync.dma_start(hit, hi_t)
            lot = cpool.tile([1, NBINS], dt.int32)
            nc.sync.dma_start(lot, lo_t)
            lastt = cpool.tile([1, NBINS], dt.int32)
            nc.sync.dma_start(lastt, last_t)
            invkt = cpool.tile([P, NBINS], dt.float32)
            nc.sync.dma_start(invkt, invk_t)
            dlinvkt = cpool.tile([P, 16], dt.float32)
            nc.sync.dma_start(dlinvkt, dlinvk_t)
            dhinvkt = cpool.tile([P, 16], dt.float32)
            nc.sync.dma_start(dhinvkt, dhinvk_t)

            xTt = xTpool.tile([P, 2 * BL * UW], dt.float32)
            xTr = xTt[:].rearrange("p (u c) -> p u c", c=UW)

            # ---------------- Phase A: fold + transpose ----------------
            with tc.tile_pool(name="xnat", bufs=2) as xnp, \
                 tc.tile_pool(name="fold", bufs=2) as fp, \
                 tc.tile_pool(name="tpps", bufs=2, space="PSUM") as tpp:
                for b in range(BL):
                    for blk in range(NBLK):
                        u = b * NBLK + blk
                        xn = xnp.tile([P, T], dt.float32, tag="xn")
                        nc.sync.dma_start(xn, x_t[b, blk * P:(blk + 1) * P, :])
                        xe = fp.tile([P, KC_E * P], dt.float32, tag="xe")
                        xo = fp.tile([P, KC_O * P], dt.float32, tag="xo")
                        nc.vector.tensor_tensor(
                            xe[:, 1:1024], xn[:, 1:1024], xn[:, 2047:1024:-1], A.add)
                        nc.vector.tensor_copy(xe[:, 0:1], xn[:, 0:1])
                        nc.vector.tensor_copy(xe[:, 1024:1025], xn[:, 1024:1025])
                        nc.vector.memset(xe[:, 1025:KC_E * P], 0.0)
                        nc.vector.tensor_tensor(
                            xo[:, 1:1024], xn[:, 1:1024], xn[:, 2047:1024:-1], A.subtract)
                        nc.vector.memset(xo[:, 0:1], 0.0)
                        for grp in range(5):
                            c0 = grp * 4
                            ncks = min(4, KCHT - c0)
                            tp = tpp.tile([P, 512], dt.float32, tag="tp")
                            for ci in range(ncks):
                                c = c0 + ci
                                src = (xe[:, c * P:(c + 1) * P] if c < KC_E
                                       else xo[:, (c - KC_E) * P:(c - KC_E + 1) * P])
                                nc.tensor.transpose(
                                    tp[:, ci * P:(ci + 1) * P], src, identt)
                            nc.scalar.activation(
                                xTt[:, u * UW + c0 * P: u * UW + c0 * P + ncks * P],
                                tp[:, 0:ncks * P], AF.Copy)

            # ---------------- Phases B+C interleaved ----------------
            # DFT runs in two 2-sample passes; each pass's moving-average work
            # is emitted immediately after it so its DVE/DMA overlaps the next
            # pass's matmuls instead of queuing behind them.
            idxrows = []
            with tc.tile_pool(name="wdma", bufs=int(os.environ.get("BUF_W", "3"))) as wp, \
                 tc.tile_pool(name="dftps", bufs=1, space="PSUM") as dpp, \
                 tc.tile_pool(name="mtps", bufs=1, space="PSUM") as mtp, \
                 tc.tile_pool(name="sq", bufs=int(os.environ.get("BUF_SQ", "2"))) as sqp, \
                 tc.tile_pool(name="xnat2", bufs=int(os.environ.get("BUF_XN", "2"))) as xnp2, \
                 tc.tile_pool(name="Gp", bufs=2) as gp, \
                 tc.tile_pool(name="colp", bufs=2) as clp, \
                 tc.tile_pool(name="magp", bufs=2) as mgp, \
                 tc.tile_pool(name="comb", bufs=int(os.environ.get("BUF_COMB", "2"))) as cbp:

                _plan = os.environ.get("KERNEL_PLAN", "2,2")
                PASSES = []           # (first_sample, n_samples)
                _s = 0
                for _n in [int(v) for v in _plan.split(",")]:
                    PASSES.append((_s, _n))
                    _s += _n
                assert _s == BL

                def emit_dft_half(half):
                    b0, SP = PASSES[half]
                    u0 = b0 * 2
                    magsum = mgp.tile([P, 8 * SP], dt.float32, tag="magsum", name="magsum")
                    for g in range(4):
                        psC = []
                        psS = []
                        for i in range(2):
                            psc_i = dpp.tile([P, 256 * SP], dt.float32, tag=f"psc{i}")
                            pss_i = dpp.tile([P, 256 * SP], dt.float32, tag=f"pss{i}")
                            psC.append(psc_i)
                            psS.append(pss_i)
                        for kc in range(KC_E):
                            wct = wp.tile([P, 2 * P], dt.float32, tag="wc")
                            nc.sync.dma_start(wct, wc_t[kc, g])
                            wst = None
                            if kc < KC_O:
                                wst = wp.tile([P, 2 * P], dt.float32, tag="ws")
                                nc.sync.dma_start(wst, ws_t[kc, g])
                            for fi in range(2):
                                rhs_e = xTr[:, u0:u0 + 2 * SP, kc * P:(kc + 1) * P]
                                nc.tensor.matmul(
                                    psC[fi], wct[:, fi * P:(fi + 1) * P], rhs_e,
                                    start=(kc == 0), stop=(kc == KC_E - 1),
                                    skip_group_check=True)
                                if kc < KC_O:
                                    rhs_o = xTr[:, u0:u0 + 2 * SP,
                                                (KC_E + kc) * P:(KC_E + kc + 1) * P]
                                    nc.tensor.matmul(
                                        psS[fi], wst[:, fi * P:(fi + 1) * P], rhs_o,
                                        start=(kc == 0), stop=(kc == KC_O - 1),
                                        skip_group_check=True)
                        for fi in range(2):
                            fc = 2 * g + fi
                            sqc = sqp.tile([P, 256 * SP], dt.float32, tag="sqc")
                            sqs = sqp.tile([P, 256 * SP], dt.float32, tag="sqs")
                            scr = sqp.tile([P, 256], dt.float32, tag="scr")
                            nc.scalar.activation(sqc, psC[fi], AF.Square)
                            nc.scalar.activation(sqs, psS[fi], AF.Square)
                            nc.vector.tensor_tensor(sqc, sqc, sqs, A.add)
                            for bh in range(SP):
                                nc.scalar.activation(
                                    scr, sqc[:, bh * 256:(bh + 1) * 256], AF.Sqrt,
                                    accum_out=magsum[:, fc * SP + bh: fc * SP + bh + 1])
                    mag_h = mgp.tile([SP, NBINS], dt.float32, tag="mag_h", name="mag_h")
                    mt = mtp.tile([8 * SP, P], dt.float32, tag="mt", name="mt")
                    nc.tensor.transpose(mt, magsum[:, 0:8 * SP], identt)
                    mtsb = mgp.tile([8 * SP, P], dt.float32, tag="mtsb", name="mtsb")
                    nc.scalar.activation(mtsb, mt, AF.Copy)
                    for fc in range(8):
                        nc.sync.dma_start(
                            mag_h[0:SP, fc * P:(fc + 1) * P],
                            mtsb[fc * SP:fc * SP + SP, :])
                    mx = mgp.tile([SP, 8], dt.float32, tag="mx", name="mx")
                    mi = mgp.tile([SP, 8], dt.uint32, tag="mi", name="mi")
                    nc.vector.max(out=mx, in_=mag_h)
                    nc.vector.max_index(mi, mx, mag_h)
                    idxrow = mgp.tile([1, 8 * SP], dt.uint32, tag="idxrow", name="idxrow")
                    nc.sync.dma_start(idxrow, mi)
                    idxrows.append(idxrow)

                def emit_sample_C(b):
                    Gs, cols = [], []
                    for blk in range(NBLK):
                        xn = xnp2.tile([P, T], dt.float32, tag="xn2", name="xn2")
                        nc.sync.dma_start(xn, x_t[b, blk * P:(blk + 1) * P, :])
                        G = gp.tile([P, GW], dt.float32, tag="G", name="G")
                        cl = clp.tile([P, 8], dt.float32, tag=f"cols{blk}",
                                      name=f"cols{blk}")
                        nc.vector.tensor_copy(cl[:, 0:1], xn[:, 0:1])
                        nc.vector.tensor_copy(cl[:, 1:2], xn[:, 2047:2048])
                        nc.vector.tensor_scalar_mul(cl[:, 2:3], cl[:, 0:1], -2049.0)
                        nc.vector.tensor_tensor_scan(
                            G[:, T + 1:2 * T + 1], xn, xn, 0.0, A.add, A.bypass)
                        nc.vector.memset(G[:, T:T + 1], 0.0)
                        nc.scalar.activation(
                            G[:, 0:T], rampt, AF.Identity,
                            bias=cl[:, 2:3], scale=cl[:, 0:1])
                        nc.scalar.activation(
                            G[:, 2 * T + 1:GW], rampt, AF.Identity,
                            bias=G[:, 2 * T:2 * T + 1], scale=cl[:, 1:2])
                        Gs.append(G)
                        cols.append(cl)
                    _half = max(h for h, (s0, _) in enumerate(PASSES) if s0 <= b)
                    _boff = b - PASSES[_half][0]
                    for kk in range(KTOP):
                        j = _boff * 8 + kk
                        _eng = (DVE + [mybir.EngineType.Pool]
                                if USE_POOL_TT else DVE)
                        idx = nc.values_load(
                            idxrows[_half][0:1, j:j + 1], engines=_eng,
                            min_val=0, max_val=NBINS - 1,
                            skip_runtime_bounds_check=True)
                        hi_s = nc.values_load(
                            hit[0:1, ds(idx, 1)], engines=_eng,
                            min_val=2065, max_val=4096,
                            skip_runtime_bounds_check=True)
                        lo_s = nc.values_load(
                            lot[0:1, ds(idx, 1)], engines=_eng,
                            min_val=1, max_val=2032,
                            skip_runtime_bounds_check=True)
                        last = nc.values_load(
                            lastt[0:1, ds(idx, 1)], engines=DVE,
                            min_val=2046, max_val=2047,
                            skip_runtime_bounds_check=True)
                        for blk in range(NBLK):
                            G, cl = Gs[blk], cols[blk]
                            comb = cbp.tile([P, T], dt.float32, tag="comb",
                                            name="comb")
                            _sub_eng = (nc.gpsimd if (USE_POOL_TT and blk == 1)
                                        else nc.vector)
                            _sub_eng.tensor_tensor(
                                comb, G[:, ds(hi_s, T)], G[:, ds(lo_s, T)],
                                A.subtract)
                            nc.vector.tensor_tensor(
                                comb[:, T - 1:T],
                                G[:, ds(hi_s + last, 1)], G[:, ds(lo_s + last, 1)],
                                A.subtract)
                            idxc = smin(idx, 15)   # dl=dh=0 for idx >= 9
                            nc.vector.tensor_scalar_mul(
                                cl[:, 4:5], cl[:, 0:1], dlinvkt[:, ds(idxc, 1)])
                            nc.vector.scalar_tensor_tensor(
                                cl[:, 5:6], cl[:, 1:2], dhinvkt[:, ds(idxc, 1)],
                                cl[:, 4:5], A.mult, A.add)
                            nc.vector.tensor_scalar(
                                comb, comb, invkt[:, ds(idx, 1)], cl[:, 5:6],
                                A.mult, A.add)
                            nc.sync.dma_start(
                                out_t[b, blk * P:(blk + 1) * P, kk, :], comb)

                for half, (s0, sp_n) in enumerate(PASSES):
                    emit_dft_half(half)
                    for bh in range(sp_n):
                        emit_sample_C(s0 + bh)

    nc.compile()
    _cache["nc"] = nc
    return nc


def _in_maps(x):
    x = np.ascontiguousarray(x, dtype=np.float32)
    return [dict(x=x[c * BL:(c + 1) * BL]) for c in range(NCORES)]


def _run(x, **kw):
    nc = _build()
    return run_bass_kernel_spmd(nc, _in_maps(x), core_ids=list(range(NCORES)), **kw)


def _get_exec():
    """Cached PJRT executable over the 8 axon cores (mirrors
    bass2jax.run_bass_via_pjrt's multi-core branch, but jit-cached)."""
    if "exec" in _cache:
        return _cache["exec"]
    import jax
    from jax.sharding import Mesh, PartitionSpec
    from jax.experimental.shard_map import shard_map
    import concourse.bass2jax as b2j
    import concourse.mybir as mybir_

    b2j.install_neuronx_cc_hook()
    nc = _build()
    pname = nc.partition_id_tensor.name if nc.partition_id_tensor else None
    in_names, out_names, out_avals, zero_shapes = [], [], [], []
    for alloc in nc.m.functions[0].allocations:
        if not isinstance(alloc, mybir_.MemoryLocationSet):
            continue
        name = alloc.memorylocations[0].name
        if alloc.kind == "ExternalInput":
            if name != pname:
                in_names.append(name)
        elif alloc.kind == "ExternalOutput":
            shape = tuple(alloc.tensor_shape)
            np_dt = mybir_.dt.np(alloc.dtype)
            out_names.append(name)
            out_avals.append(jax.core.ShapedArray(shape, np_dt))
            zero_shapes.append((shape, np_dt))
    n_params = len(in_names)
    all_in_names = in_names + out_names
    if pname is not None:
        all_in_names = all_in_names + [pname]

    def _body(*args):
        operands = list(args)
        if pname is not None:
            operands.append(b2j.partition_id_tensor())
        outs = b2j._bass_exec_p.bind(
            *operands,
            out_avals=tuple(out_avals),
            in_names=tuple(all_in_names),
            out_names=tuple(out_names),
            lowering_input_output_aliases=(),
            sim_require_finite=True,
            sim_require_nnan=True,
            nc=nc,
        )
        return tuple(outs)

    devices = jax.devices()[:NCORES]
    mesh = Mesh(np.asarray(devices), ("core",))
    nio = n_params + len(out_names)
    sharded = jax.jit(
        shard_map(_body, mesh=mesh,
                  in_specs=(PartitionSpec("core"),) * nio,
                  out_specs=(PartitionSpec("core"),) * len(out_names),
                  check_rep=False),
        donate_argnums=tuple(range(n_params, nio)),
        keep_unused=True,
    )
    ex = dict(sharded=sharded, in_names=in_names, out_names=out_names,
              out_avals=out_avals, zero_shapes=zero_shapes, mesh=mesh)
    _cache["exec"] = ex
    return ex


def _concat_inputs(x):
    ex = _get_exec()
    maps = _in_maps(x)
    return [np.concatenate([maps[c][n] for c in range(NCORES)], axis=0)
            for n in ex["in_names"]]


def _make_zeros(on_device=False):
    ex = _get_exec()
    if on_device:
        import jax.numpy as jnp
        from jax.sharding import NamedSharding, PartitionSpec
        sh = NamedSharding(ex["mesh"], PartitionSpec("core"))
        return [jnp.zeros((NCORES * s[0], *s[1:]), d, device=sh)
                for s, d in ex["zero_shapes"]]
    return [np.zeros((NCORES * s[0], *s[1:]), d) for s, d in ex["zero_shapes"]]


def kernel(x):
    try:
        ex = _get_exec()
        outs = ex["sharded"](*_concat_inputs(x), *_make_zeros())
        out = np.asarray(outs[ex["out_names"].index("out")])
        return out.reshape(NCORES, BL, N, KTOP, T).reshape(B, N, KTOP, T)
    except Exception:
        res = _run(x)
        return np.concatenate([res.results[c]["out"] for c in range(NCORES)],
                              axis=0)

